# revision 6
# baseline (speedup 1.0000x reference)
"""Trainium2 Bass kernel for nn_MultiHeadAttention_50972671869112.

Reference computation (per batch b):
    q = (Q @ Wq + bq)  -> heads [H, S, DK]
    k = (K @ Wk + bk) + pos_k[h]
    v = (V @ Wv + bv) + pos_v[h]
    attn = softmax(q @ k.T / sqrt(DK))
    ctx  = attn @ v
    out  = layernorm(ctx @ Wo + bo + Q)
    returns (out, attn)

Sharding: 8 cores = (batch b, query-half qh).  Each core owns 512 query rows of
one batch and all 16 heads.  K/V projections are recomputed per core pair
(cheap vs. collectives).  All matmuls run as float32r (full-rate fp32 path).

Device layouts (T = transposed, hd = h*64+d):
    qT [hd, qrow]   : lhsT=Wq chunk, rhs=Q[b,rows].T          (scale 1/8 folded into Wq,bq)
    kT [hd, krow]   : lhsT=Wk chunk, rhs=K[b].T  + pos_k arranged [hd, s] (bk folded)
    v  [krow, h, 65]: lhsT=V[b].T chunk, rhs=Wv  + pos_v arranged [s, hd] (bv folded);
                      col 64 = ones -> context matmul also yields softmax row-sums
    scoresT [krow, qrow] per head = kT_h.T-slice x qT_h   (K=64 row-tiled matmuls)
    p = exp(scoresT)  (max-subtraction skipped; |scores| < ~8 so exp is safe in fp32)
    ctxT+sum [65, qrow] = v_aug.T @ p
    attnT = p * (1/rowsum)  -> HBM [h, k, q]; host transposes to [h, q, k]
    out = ctxT.T @ Wo(rearranged [dk, h, d]) + bo + Qres, then layernorm rows.
"""

import numpy as np

import concourse.bass as bass
import concourse.mybir as mybir
import concourse.tile as tile
from concourse import bacc, bass_utils
from concourse.bass import ts

F32 = mybir.dt.float32
F32R = mybir.dt.float32r
AF = mybir.ActivationFunctionType

P = 128
B, S, D = 4, 1024, 1024
H, DK, DV = 16, 64, 64
QR = 512          # query rows per core
NCORES = 8
NDC = D // P      # contraction chunks over model dim
NHC = D // P      # hd chunks (2 heads per chunk)
NKC = S // P      # key-row chunks
EPS = 1e-5

# Set False to fall back to exact (4x slower) fp32 matmuls.
USE_F32R = True


MMDT = F32R if USE_F32R else F32


def _mm(ap):
    return ap


def build_nc():
    nc = bacc.Bacc("TRN2", target_bir_lowering=False, debug=False,
                   num_devices=NCORES)

    qt = nc.dram_tensor("qt", [D, QR], MMDT, kind="ExternalInput")
    qres = nc.dram_tensor("qres", [QR, D], F32, kind="ExternalInput")
    kt = nc.dram_tensor("kt", [D, S], MMDT, kind="ExternalInput")
    vt = nc.dram_tensor("vt", [D, S], MMDT, kind="ExternalInput")
    wq = nc.dram_tensor("wq", [D, D], MMDT, kind="ExternalInput")
    wk = nc.dram_tensor("wk", [D, D], MMDT, kind="ExternalInput")
    wv = nc.dram_tensor("wv", [D, D], MMDT, kind="ExternalInput")
    wo = nc.dram_tensor("wo", [DK, H, D], MMDT, kind="ExternalInput")
    bqv = nc.dram_tensor("bqv", [P, NHC], F32, kind="ExternalInput")
    poskb = nc.dram_tensor("poskb", [D, S], F32, kind="ExternalInput")
    posvb = nc.dram_tensor("posvb", [S, D], F32, kind="ExternalInput")
    bo = nc.dram_tensor("bo", [D], F32, kind="ExternalInput")
    gamma = nc.dram_tensor("gamma", [D], F32, kind="ExternalInput")
    beta = nc.dram_tensor("beta", [D], F32, kind="ExternalInput")

    attn_t = nc.dram_tensor("attn_t", [H, S, QR], F32, kind="ExternalOutput")
    out_s = nc.dram_tensor("out_s", [QR, D], F32, kind="ExternalOutput")

    with tile.TileContext(nc) as tc:
        # Outermost pool: lives for the whole kernel (ctxsb bridges B -> C).
        with tc.tile_pool(name="persist", bufs=1) as persist:
            ctxsb = persist.tile([DK, H, QR], MMDT, name="ctxsb")
            bq_sb = persist.tile([P, NHC], F32, name="bq_sb")
            nc.sync.dma_start(out=bq_sb, in_=bqv[:, :])

            # qT/kT/v live through phases A+B only.
            with tc.tile_pool(name="qkv_res", bufs=1) as qkvp:
                qT = [qkvp.tile([P, QR], MMDT, name=f"qT{i}") for i in range(NHC)]
                kT = [qkvp.tile([P, S], MMDT, name=f"kT{i}") for i in range(NHC)]
                vsb = [qkvp.tile([P, H, DK + 1], MMDT, name=f"vsb{i}")
                       for i in range(NKC)]
                for i in range(NKC):
                    nc.vector.memset(vsb[i][:, :, DK:DK + 1].bitcast(F32), 1.0)

                # ---------------- Phase A1: q projection ----------------
                with tc.tile_pool(name="a1w", bufs=10) as a1w, \
                     tc.tile_pool(name="a1a", bufs=9) as a1a, \
                     tc.tile_pool(name="psA1", bufs=4, space="PSUM") as psA1:
                    wq_t = []
                    qt_t = []
                    for dc in range(NDC):
                        w = a1w.tile([P, D], MMDT, tag="w", name=f"wq{dc}")
                        nc.sync.dma_start(out=w, in_=wq[ts(dc, P), :])
                        wq_t.append(w)
                        a = a1a.tile([P, QR], MMDT, tag="a", name=f"qt{dc}")
                        nc.sync.dma_start(out=a, in_=qt[ts(dc, P), :])
                        qt_t.append(a)
                    for hc in range(NHC):
                        ps = psA1.tile([P, QR], F32, tag="ps", name=f"psq{hc}")
                        for dc in range(NDC):
                            nc.tensor.matmul(
                                ps,
                                lhsT=_mm(wq_t[dc][:, ts(hc, P)]),
                                rhs=_mm(qt_t[dc]),
                                start=(dc == 0),
                                stop=(dc == NDC - 1),
                            )
                        nc.scalar.activation(
                            out=qT[hc], in_=ps, func=AF.Identity,
                            bias=bq_sb[:, hc:hc + 1],
                        )

                # ---------------- Phase A2: k and v projections ----------------
                with tc.tile_pool(name="a2w", bufs=9) as a2w, \
                     tc.tile_pool(name="a2a", bufs=9) as a2a, \
                     tc.tile_pool(name="a2p", bufs=2) as a2p, \
                     tc.tile_pool(name="psA2", bufs=4, space="PSUM") as psA2:
                    wk_t = []
                    kt_t = []
                    for dc in range(NDC):
                        w = a2w.tile([P, D], MMDT, tag="w", name=f"wk{dc}")
                        nc.sync.dma_start(out=w, in_=wk[ts(dc, P), :])
                        wk_t.append(w)
                        a = a2a.tile([P, S], MMDT, tag="a", name=f"kt{dc}")
                        nc.sync.dma_start(out=a, in_=kt[ts(dc, P), :])
                        kt_t.append(a)
                    for hc in range(NHC):
                        pkb = a2p.tile([P, S], F32, tag="pos", name=f"pkb{hc}")
                        nc.sync.dma_start(out=pkb, in_=poskb[ts(hc, P), :])
                        for sc in range(2):
                            ps = psA2.tile([P, QR], F32, tag="ps",
                                           name=f"psk{hc}_{sc}")
                            for dc in range(NDC):
                                nc.tensor.matmul(
                                    ps,
                                    lhsT=_mm(wk_t[dc][:, ts(hc, P)]),
                                    rhs=_mm(kt_t[dc][:, ts(sc, QR)]),
                                    start=(dc == 0),
                                    stop=(dc == NDC - 1),
                                )
                            nc.vector.tensor_add(
                                out=kT[hc][:, ts(sc, QR)], in0=ps,
                                in1=pkb[:, ts(sc, QR)],
                            )

                    wv_t = []
                    vt_t = []
                    for dc in range(NDC):
                        w = a2w.tile([P, D], MMDT, tag="w", name=f"wv{dc}")
                        nc.sync.dma_start(out=w, in_=wv[ts(dc, P), :])
                        wv_t.append(w)
                        a = a2a.tile([P, S], MMDT, tag="a", name=f"vt{dc}")
                        nc.sync.dma_start(out=a, in_=vt[ts(dc, P), :])
                        vt_t.append(a)
                    for kc in range(NKC):
                        pvb = a2p.tile([P, D], F32, tag="pos", name=f"pvb{kc}")
                        nc.sync.dma_start(out=pvb, in_=posvb[ts(kc, P), :])
                        for hh in range(2):
                            ps = psA2.tile([P, QR], F32, tag="ps",
                                           name=f"psv{kc}_{hh}")
                            for dc in range(NDC):
                                nc.tensor.matmul(
                                    ps,
                                    lhsT=_mm(vt_t[dc][:, ts(kc, P)]),
                                    rhs=_mm(wv_t[dc][:, ts(hh, QR)]),
                                    start=(dc == 0),
                                    stop=(dc == NDC - 1),
                                )
                            nc.vector.tensor_add(
                                out=vsb[kc][:, hh * 8:(hh + 1) * 8, 0:DK],
                                in0=ps.rearrange("p (h d) -> p h d", d=DK),
                                in1=pvb[:, ts(hh, QR)].rearrange(
                                    "p (h d) -> p h d", d=DK),
                            )

                # ---------------- Phase B: attention per head ----------------
                with tc.tile_pool(name="ppool", bufs=18) as ppool, \
                     tc.tile_pool(name="atpool", bufs=6) as atpool, \
                     tc.tile_pool(name="rbpool", bufs=4) as rbpool, \
                     tc.tile_pool(name="drpool", bufs=4, space="DRAM") as drpool, \
                     tc.tile_pool(name="psS", bufs=5, space="PSUM") as psS, \
                     tc.tile_pool(name="psC", bufs=2, space="PSUM") as psC:
                    for h in range(H):
                        hc, off = h // 2, (h % 2) * DK
                        pts = []
                        for kc in range(NKC):
                            ps = psS.tile([P, QR], F32, tag="s",
                                          name=f"pss{h}_{kc}")
                            nc.tensor.matmul(
                                ps,
                                lhsT=_mm(kT[hc][off:off + DK, ts(kc, P)]),
                                rhs=_mm(qT[hc][off:off + DK, :]),
                                start=True,
                                stop=True,
                            )
                            pt = ppool.tile([P, QR], MMDT, tag="pt",
                                            name=f"pt{h}_{kc}")
                            nc.scalar.activation(out=pt, in_=ps, func=AF.Exp)
                            pts.append(pt)
                        psc = psC.tile([P, QR], F32, tag="c", name=f"psc{h}")
                        for kc in range(NKC):
                            nc.tensor.matmul(
                                psc[0:DK + 1, :],
                                lhsT=_mm(vsb[kc][:, h, :]),
                                rhs=_mm(pts[kc]),
                                start=(kc == 0),
                                stop=(kc == NKC - 1),
                            )
                        recip = rbpool.tile([1, QR], F32, tag="r",
                                            name=f"recip{h}")
                        nc.vector.reciprocal(out=recip, in_=psc[DK:DK + 1, :])
                        rd = drpool.tile([1, QR], F32, tag="rd", name=f"rd{h}")
                        nc.sync.dma_start(out=rd, in_=recip)
                        rb = rbpool.tile([P, QR], F32, tag="rb", name=f"rb{h}")
                        nc.sync.dma_start(out=rb, in_=rd.to_broadcast((P, QR)))
                        nc.vector.tensor_mul(
                            out=ctxsb[:, h, :], in0=psc[0:DK, :], in1=rb[0:DK, :],
                        )
                        for kc in range(NKC):
                            at = atpool.tile([P, QR], F32, tag="at",
                                             name=f"at{h}_{kc}")
                            nc.vector.tensor_mul(out=at, in0=pts[kc], in1=rb)
                            nc.sync.dma_start(out=attn_t[h, ts(kc, P), :], in_=at)

            # qkv_res released here; ctxsb still live.
            # ------------- Phase C: output projection + layernorm -------------
            with tc.tile_pool(name="cconst", bufs=1) as ccp, \
                 tc.tile_pool(name="wop", bufs=3) as wop, \
                 tc.tile_pool(name="cpool", bufs=3) as cpool, \
                 tc.tile_pool(name="psO", bufs=8, space="PSUM") as psO:
                bo_b = ccp.tile([P, D], F32, name="bo_b")
                nc.sync.dma_start(out=bo_b, in_=bo[None, :].to_broadcast((P, D)))
                g_b = ccp.tile([P, D], F32, name="g_b")
                nc.sync.dma_start(out=g_b, in_=gamma[None, :].to_broadcast((P, D)))
                be_b = ccp.tile([P, D], F32, name="be_b")
                nc.sync.dma_start(out=be_b, in_=beta[None, :].to_broadcast((P, D)))
                eps_t = ccp.tile([P, 1], F32, name="eps_t")
                nc.vector.memset(eps_t, EPS)

                pso = [
                    [psO.tile([P, QR], F32, tag="o", name=f"pso{q4}_{nh}")
                     for nh in range(2)]
                    for q4 in range(QR // P)
                ]
                for h in range(H):
                    wot = wop.tile([DK, D], MMDT, tag="wo", name=f"wot{h}")
                    nc.sync.dma_start(out=wot, in_=wo[:, h, :])
                    for q4 in range(QR // P):
                        for nh in range(2):
                            nc.tensor.matmul(
                                pso[q4][nh],
                                lhsT=_mm(ctxsb[:, h, ts(q4, P)]),
                                rhs=_mm(wot[:, ts(nh, QR)]),
                                start=(h == 0),
                                stop=(h == H - 1),
                            )
                for q4 in range(QR // P):
                    qr_t = cpool.tile([P, D], F32, tag="qr", name=f"qr{q4}")
                    nc.sync.dma_start(out=qr_t, in_=qres[ts(q4, P), :])
                    o_t = cpool.tile([P, D], F32, tag="o", name=f"o{q4}")
                    for nh in range(2):
                        nc.vector.tensor_add(
                            out=o_t[:, ts(nh, QR)], in0=pso[q4][nh],
                            in1=bo_b[:, ts(nh, QR)],
                        )
                    nc.vector.tensor_add(out=o_t, in0=o_t, in1=qr_t)
                    stats = cpool.tile([P, 2, 6], F32, tag="st", name=f"st{q4}")
                    for sg in range(2):
                        nc.vector.bn_stats(out=stats[:, sg, :],
                                           in_=o_t[:, ts(sg, QR)])
                    mv = cpool.tile([P, 2], F32, tag="mv", name=f"mv{q4}")
                    nc.vector.bn_aggr(out=mv, in_=stats)
                    std = cpool.tile([P, 1], F32, tag="sd", name=f"sd{q4}")
                    nc.scalar.activation(
                        out=std, in_=mv[:, 1:2], func=AF.Sqrt, bias=eps_t,
                    )
                    nc.vector.reciprocal(out=std, in_=std)
                    o_n = cpool.tile([P, D], F32, tag="on", name=f"on{q4}")
                    nc.vector.tensor_scalar(
                        out=o_n, in0=o_t, scalar1=mv[:, 0:1], scalar2=std,
                        op0=mybir.AluOpType.subtract, op1=mybir.AluOpType.mult,
                    )
                    nc.vector.tensor_mul(out=o_n, in0=o_n, in1=g_b)
                    nc.vector.tensor_add(out=o_n, in0=o_n, in1=be_b)
                    nc.sync.dma_start(out=out_s[ts(q4, P), :], in_=o_n)

    nc.compile()
    return nc


_NC = None


def _get_nc():
    global _NC
    if _NC is None:
        _NC = build_nc()
    return _NC


def make_in_maps(inputs):
    f = lambda x: np.ascontiguousarray(np.asarray(x), dtype=np.float32)
    Q, K, V = f(inputs["Q"]), f(inputs["K"]), f(inputs["V"])
    Wq, bq = f(inputs["Wq"]), f(inputs["bq"])
    Wk, bk = f(inputs["Wk"]), f(inputs["bk"])
    Wv, bv = f(inputs["Wv"]), f(inputs["bv"])
    Wo, bo = f(inputs["Wo"]), f(inputs["bo"])
    pos_k, pos_v = f(inputs["pos_k"]), f(inputs["pos_v"])
    gamma, beta = f(inputs["gamma"]), f(inputs["beta"])

    sc = np.float32(1.0) / np.sqrt(np.float32(DK))
    wq_s = np.ascontiguousarray(Wq * sc)
    # bq arranged [p, o] with hd = o*128 + p, pre-scaled
    bq_arr = np.ascontiguousarray((bq * sc).reshape(NHC, P).T)
    # pos_k (+bk) arranged [hd, s]
    poskb = np.ascontiguousarray(
        (pos_k.reshape(H, S, DK) + bk.reshape(H, 1, DK))
        .transpose(0, 2, 1).reshape(H * DK, S)
    )
    # pos_v (+bv) arranged [s, hd]
    posvb = np.ascontiguousarray(
        (pos_v.reshape(H, S, DV) + bv.reshape(H, 1, DV))
        .transpose(1, 0, 2).reshape(S, H * DV)
    )
    wo_r = np.ascontiguousarray(Wo.reshape(H, DV, D).transpose(1, 0, 2))

    in_maps = []
    for c in range(NCORES):
        b, qh = c // 2, c % 2
        rows = slice(qh * QR, (qh + 1) * QR)
        in_maps.append(dict(
            qt=np.ascontiguousarray(Q[b, rows, :].T),
            qres=np.ascontiguousarray(Q[b, rows, :]),
            kt=np.ascontiguousarray(K[b].T),
            vt=np.ascontiguousarray(V[b].T),
            wq=wq_s, wk=Wk, wv=Wv, wo=wo_r,
            bqv=bq_arr, poskb=poskb, posvb=posvb,
            bo=bo, gamma=gamma, beta=beta,
        ))
    return in_maps


def assemble(per_core):
    out = np.empty((B, S, D), np.float32)
    attn = np.empty((B, H, S, S), np.float32)
    for c in range(NCORES):
        b, qh = c // 2, c % 2
        r = per_core[c]
        out[b, qh * QR:(qh + 1) * QR, :] = r["out_s"]
        attn[b, :, qh * QR:(qh + 1) * QR, :] = r["attn_t"].transpose(0, 2, 1)
    return out, attn


TRACE = False
LAST_RESULTS = None


def kernel(**inputs):
    global LAST_RESULTS
    nc = _get_nc()
    in_maps = make_in_maps(inputs)
    res = bass_utils.run_bass_kernel_spmd(
        nc, in_maps, core_ids=list(range(NCORES)), trace=TRACE,
    )
    LAST_RESULTS = res
    return assemble(res.results)
